# revision 1
# baseline (speedup 1.0000x reference)
"""DeformConv2d (DCNv2) Trainium2 Bass kernel.

Problem: N=4, C_IN=C_OUT=64, H=W=128, 3x3 taps, stride=1, pad=1, dil=1,
modulated deformable conv (torchvision semantics).

Sharding: 8 cores; core = (image n = core//2, row-half = core%2).
Each core computes out[n, :, i0:i0+64, :] from the full image x[n].

Per-core pipeline (all arithmetic on device):
  1. DVE: frac/floor of offsets via AluOpType.mod, bilinear corner weights
     (modulation mask folded in), int16 gather indices built against a
     constant affine index ramp.
  2. Pool/SWDGE: dma_gather pair-mode from zero-padded NHWC image in DRAM
     (elem = 2 pixels x 64ch x fp32 = 512B per descriptor; y0/y1 rows are
     two descriptors).  Zero padding makes out-of-bounds corners exact.
  3. DVE: weighted 4-corner combine using stride-0 broadcast weight APs.
  4. PE: per-row transposes [128j, 64c] -> [64c, 128j], then 9 accumulating
     matmuls (contraction c=64) with weight slices as stationaries.
"""
import sys
import os

_TRN_REPO = "/opt/trn_rl_repo"
if _TRN_REPO not in sys.path:
    sys.path.insert(0, _TRN_REPO)

import numpy as np

import concourse.bass as bass
import concourse.bacc as bacc
import concourse.tile as tile
import concourse.mybir as mybir
from concourse import library_config
from concourse.bass_utils import run_bass_kernel_spmd
from contextlib import ExitStack

F32 = mybir.dt.float32
I16 = mybir.dt.int16
ALU = mybir.AluOpType

N, C, H, W = 4, 64, 128, 128
K2 = 9
PAD = 16                    # coordinate padding on each side
PH = H + 2 * PAD            # 160
PW = W + 2 * PAD            # 160
NROWS = PH * PW             # 25600 pixel-rows of 64 channels in padded image
HI = 64                     # rows per core
R = 16                      # rows per block
NBLK = HI // R              # 4
NIDX_BLK = R * 2 * W        # 4096 gather descriptors per (k, block)
CLAMP = 11.0                # |floor(offset)| clamp (pad-region safe)

_CACHED = {}


def build_nc():
    nc = bacc.Bacc(trn_type="TRN2", debug=False, num_swdge_queues=4)

    xp_d = nc.dram_tensor("xp", [NROWS * C], F32, kind="ExternalInput")
    offj_d = nc.dram_tensor("offj", [128, 2 * K2 * HI], F32, kind="ExternalInput").ap()
    maskj_d = nc.dram_tensor("maskj", [128, K2 * HI], F32, kind="ExternalInput").ap()
    idxb_d = nc.dram_tensor("idxb", [128, K2 * HI * 2 * 8], F32, kind="ExternalInput").ap()
    wk_d = nc.dram_tensor("wk", [64, K2 * 64], F32, kind="ExternalInput").ap()
    ident_d = nc.dram_tensor("ident", [128, 128], F32, kind="ExternalInput").ap()
    out_d = nc.dram_tensor("out", [64, HI * W], F32, kind="ExternalOutput").ap()
    scr_d = nc.dram_tensor("dyx_scratch", [128 * K2 * HI], F32)

    # gather source: overlapping pixel-pair rows of the padded image
    src_ap = bass.AP(xp_d, 0, [[C, NROWS - 1], [1, 2 * C]])

    with ExitStack() as ctx:
        tc = ctx.enter_context(tile.TileContext(nc))

        const = ctx.enter_context(tc.tile_pool(name="const", bufs=1))
        # live across phase 2: idxs + w4
        live = ctx.enter_context(tc.tile_pool(name="live", bufs=1))
        scratch_ctx = ExitStack()
        work = scratch_ctx.enter_context(tc.tile_pool(name="work", bufs=1))

        offj = const.tile([128, 2 * K2 * HI], F32)
        nc.sync.dma_start(offj[:], offj_d)
        maskj = const.tile([128, K2 * HI], F32)
        nc.sync.dma_start(maskj[:], maskj_d)
        idxb = const.tile([128, K2 * HI * 2 * 8], F32)
        nc.sync.dma_start(idxb[:], idxb_d)
        wk = const.tile([64, K2 * 64], F32)
        nc.sync.dma_start(wk[:], wk_d)
        ident = const.tile([128, 128], F32)
        nc.sync.dma_start(ident[:], ident_d)

        # ---- Phase 1: frac / floor / weights / indices -------------------
        # floor via round-to-nearest magic constant: rne(x) = (x + M) - M,
        # floor(x) = rne(x) - (rne(x) > x); frac = x - floor(x).  Exact for
        # |x| < 2^22 in fp32; avoids AluOpType.mod (not in DVE ISA).
        MAGIC = 12582912.0  # 1.5 * 2**23
        flo = work.tile([128, 2 * K2 * HI], F32)
        nc.vector.tensor_scalar(flo[:], offj[:], MAGIC, None, ALU.add)
        nc.vector.tensor_scalar(flo[:], flo[:], MAGIC, None, ALU.subtract)
        rup = work.tile([128, 2 * K2 * HI], F32)
        nc.vector.tensor_tensor(rup[:], flo[:], offj[:], ALU.is_gt)
        nc.vector.tensor_tensor(flo[:], flo[:], rup[:], ALU.subtract)
        frac = work.tile([128, 2 * K2 * HI], F32)
        nc.vector.tensor_tensor(frac[:], offj[:], flo[:], ALU.subtract)
        nc.vector.tensor_scalar(flo[:], flo[:], -CLAMP, None, ALU.max)
        nc.vector.tensor_scalar(flo[:], flo[:], CLAMP, None, ALU.min)

        # offj channel layout: ch = 2k (dy), 2k+1 (dx); free = (ch, i)
        def kv(t):  # [128, (k, two, i)]
            return t[:].rearrange("p (k two i) -> p k two i", k=K2, two=2, i=HI)

        dyx = work.tile([128, K2 * HI], F32)
        dyx3 = dyx[:].rearrange("p (k i) -> p k i", k=K2, i=HI)
        nc.vector.tensor_scalar(dyx3, kv(flo)[:, :, 0, :], float(PW), None, ALU.mult)
        nc.vector.tensor_tensor(dyx3, dyx3, kv(flo)[:, :, 1, :], ALU.add)

        # repack dyx [j, (k,i)] -> dyx_w [16q+u, (k,i,jw)] via DRAM bounce
        nc.sync.dma_start(bass.AP(scr_d, 0, [[K2 * HI, 128], [1, K2 * HI]]), dyx[:])
        dyx_w = work.tile([128, K2 * HI * 8], F32)
        for q in range(8):
            # dst partition 16q+u, free (k, i, jw); src scratch[(16*jw+u)*576 + k*64 + i]
            nc.sync.dma_start(
                dyx_w[16 * q:16 * q + 16, :].rearrange(
                    "p (k i jw) -> p k i jw", k=K2, i=HI, jw=8),
                bass.AP(scr_d, 0,
                        [[K2 * HI, 16], [64, K2], [1, HI], [16 * K2 * HI, 8]]),
            )

        # idxs[p, (k, i, yc, jw)] = idxb + dyx_w (broadcast over yc)
        idxs = live.tile([128, K2 * HI * 2 * 8], I16)
        dyx_b = bass.AP(
            dyx_w[:].tensor, dyx_w[:].offset,
            [dyx_w[:].ap[0], [8, K2 * HI], [0, 2], [1, 8]],
        )
        nc.vector.tensor_tensor(
            idxs[:].rearrange("p (ki yc jw) -> p ki yc jw",
                              ki=K2 * HI, yc=2, jw=8),
            idxb[:].rearrange("p (ki yc jw) -> p ki yc jw",
                              ki=K2 * HI, yc=2, jw=8),
            dyx_b, ALU.add)

        # corner weights w4[j, (k, i, yc, xc)]
        fr = kv(frac)
        wy = fr[:, :, 0, :]            # [128, k, i]
        wx = fr[:, :, 1, :]
        omy = work.tile([128, K2 * HI], F32)
        omyv = omy[:].rearrange("p (k i) -> p k i", k=K2, i=HI)
        nc.vector.tensor_scalar(omyv, wy, 1.0, None, ALU.subtract)
        nc.vector.tensor_scalar(omyv, omyv, -1.0, None, ALU.mult)
        omx = work.tile([128, K2 * HI], F32)
        omxv = omx[:].rearrange("p (k i) -> p k i", k=K2, i=HI)
        nc.vector.tensor_scalar(omxv, wx, 1.0, None, ALU.subtract)
        nc.vector.tensor_scalar(omxv, omxv, -1.0, None, ALU.mult)
        m3 = maskj[:].rearrange("p (k i) -> p k i", k=K2, i=HI)
        wxm0 = work.tile([128, K2 * HI], F32)
        nc.vector.tensor_tensor(
            wxm0[:].rearrange("p (k i) -> p k i", k=K2, i=HI),
            omx[:].rearrange("p (k i) -> p k i", k=K2, i=HI), m3, ALU.mult)
        wxm1 = work.tile([128, K2 * HI], F32)
        nc.vector.tensor_tensor(
            wxm1[:].rearrange("p (k i) -> p k i", k=K2, i=HI), wx, m3, ALU.mult)

        w4 = live.tile([128, K2 * HI * 4], F32)
        w4v = w4[:].rearrange("p (k i yc xc) -> p k i yc xc",
                              k=K2, i=HI, yc=2, xc=2)
        omy3 = omy[:].rearrange("p (k i) -> p k i", k=K2, i=HI)
        wy3 = wy
        wxm0v = wxm0[:].rearrange("p (k i) -> p k i", k=K2, i=HI)
        wxm1v = wxm1[:].rearrange("p (k i) -> p k i", k=K2, i=HI)
        nc.vector.tensor_tensor(w4v[:, :, :, 0, 0], omy3, wxm0v, ALU.mult)
        nc.vector.tensor_tensor(w4v[:, :, :, 0, 1], omy3, wxm1v, ALU.mult)
        nc.vector.tensor_tensor(w4v[:, :, :, 1, 0], wy3, wxm0v, ALU.mult)
        nc.vector.tensor_tensor(w4v[:, :, :, 1, 1], wy3, wxm1v, ALU.mult)

        # ---- Phase 2: gather / combine / transpose / conv ----------------
        scratch_ctx.close()
        gpool = ctx.enter_context(tc.tile_pool(name="g", bufs=2))
        p4pool = ctx.enter_context(tc.tile_pool(name="p4", bufs=2))
        s2pool = ctx.enter_context(tc.tile_pool(name="s2", bufs=2))
        spool = ctx.enter_context(tc.tile_pool(name="s", bufs=2))
        stpool = ctx.enter_context(tc.tile_pool(name="st", bufs=2))
        obpool = ctx.enter_context(tc.tile_pool(name="ob", bufs=2))
        tpps = ctx.enter_context(tc.tile_pool(name="tp", bufs=2, space="PSUM"))
        outps = ctx.enter_context(tc.tile_pool(name="ops", bufs=1, space="PSUM"))

        idxs5 = idxs[:].rearrange("p (k i yc jw) -> p k i yc jw",
                                  k=K2, i=HI, yc=2, jw=8)
        w4_5 = w4[:].rearrange("p (k i yc xc) -> p k i yc xc",
                               k=K2, i=HI, yc=2, xc=2)

        for b in range(NBLK):
            out_ps = outps.tile([64, R * W], F32)
            for k in range(K2):
                g = gpool.tile([128, R * 2 * 128], F32)
                # SWDGE ring holds 1024 descriptors -> 4 rows (1024 idxs)
                # per dma_gather call, round-robined over 4 queues.
                RSUB = 4
                gv = g[:].rearrange("p (s e) -> p s e", s=R * 2, e=128)
                for sub in range(R // RSUB):
                    nidx = RSUB * 2 * 128
                    nc.gpsimd.dma_gather(
                        gv[:, sub * RSUB * 2:(sub + 1) * RSUB * 2, :],
                        src_ap,
                        idxs5[:, k, b * R + sub * RSUB:b * R + (sub + 1) * RSUB, :, :],
                        nidx,
                        nidx,
                        elem_size=2 * C,
                        elem_step=C,
                        queue_num=(b * K2 * (R // RSUB) + k * (R // RSUB) + sub) % 4,
                    )
                # weighted corners
                p4 = p4pool.tile([128, R * 2 * 2 * C], F32)
                wsl = w4_5[:, k, b * R:(b + 1) * R, :, :]
                w_b = bass.AP(
                    wsl.tensor, wsl.offset,
                    [wsl.ap[0], [4, R], [1, 4], [0, C]],
                )
                nc.vector.tensor_tensor(
                    p4[:].rearrange("p (i cr c) -> p i cr c", i=R, cr=4, c=C),
                    g[:].rearrange("p (i cr c) -> p i cr c", i=R, cr=4, c=C),
                    w_b, ALU.mult)
                # sum y-corners, then x-corners
                s2 = s2pool.tile([128, R * 2 * C], F32)
                p4v = p4[:].rearrange("p (i yc cc) -> p i yc cc",
                                      i=R, yc=2, cc=2 * C)
                nc.vector.tensor_tensor(
                    s2[:].rearrange("p (i cc) -> p i cc", i=R, cc=2 * C),
                    p4v[:, :, 0, :], p4v[:, :, 1, :], ALU.add)
                s = spool.tile([128, R * C], F32)
                s2v = s2[:].rearrange("p (i xc c) -> p i xc c", i=R, xc=2, c=C)
                sv = s[:].rearrange("p (i c) -> p i c", i=R, c=C)
                nc.vector.tensor_tensor(
                    sv, s2v[:, :, 0, :], s2v[:, :, 1, :], ALU.add)
                # transpose to [c, (i, j)] and conv-accumulate
                st = stpool.tile([64, R * 128], F32)
                for h in range(R // 8):
                    tp = tpps.tile([64, 8 * 128], F32)
                    for i2 in range(8):
                        i = h * 8 + i2
                        nc.tensor.transpose(
                            tp[:, i2 * 128:(i2 + 1) * 128], sv[:, i, :], ident[:])
                    nc.scalar.copy(
                        st[:, h * 8 * 128:(h + 1) * 8 * 128], tp[:])
                for c4 in range(R * W // 512):
                    nc.tensor.matmul(
                        out_ps[:, c4 * 512:(c4 + 1) * 512],
                        wk[:, k * 64:(k + 1) * 64],
                        st[:, c4 * 512:(c4 + 1) * 512],
                        start=(k == 0), stop=(k == K2 - 1))
            ob = obpool.tile([64, R * W], F32)
            nc.scalar.copy(ob[:], out_ps[:])
            nc.sync.dma_start(out_d[:, b * R * W:(b + 1) * R * W], ob[:])

    if not nc.is_finalized():
        nc.finalize()
    return nc


def _prep_core(x, offset, mask, weight_kco, core):
    n, half = core // 2, core % 2
    i0 = half * HI
    xp = np.zeros((PH, PW, C), np.float32)
    xp[PAD:PAD + H, PAD:PAD + W, :] = x[n].transpose(1, 2, 0)
    offj = np.ascontiguousarray(
        offset[n, :, i0:i0 + HI, :].transpose(2, 0, 1)).reshape(128, 2 * K2 * HI)
    maskj = np.ascontiguousarray(
        mask[n, :, i0:i0 + HI, :].transpose(2, 0, 1)).reshape(128, K2 * HI)

    u = (np.arange(128) % 16).astype(np.int64)
    k = np.arange(K2)
    ki, kj = k // 3, k % 3
    i = np.arange(HI)
    yc = np.arange(2)
    jw = np.arange(8)
    base = ((i0 + i[None, :, None, None] + ki[:, None, None, None] - 1 + PAD
             + yc[None, None, :, None]) * PW
            + jw[None, None, None, :] * 16 + kj[:, None, None, None] - 1 + PAD)
    idxb = (base[None] + u[:, None, None, None, None]).reshape(128, -1)
    assert idxb.min() >= -CLAMP * PW - CLAMP and idxb.max() + CLAMP * PW + CLAMP + PW < NROWS
    idxb = idxb.astype(np.float32)

    return {
        "xp": xp.reshape(-1),
        "offj": offj,
        "maskj": maskj,
        "idxb": idxb,
        "wk": weight_kco,
        "ident": np.eye(128, dtype=np.float32),
    }


def kernel_traced(x, offset, mask, weight, trace=True, trace_kwargs=None):
    """Like kernel() but runs with NTFF tracing; returns (out, BassKernelResults)."""
    x = np.asarray(x, np.float32)
    offset = np.asarray(offset, np.float32)
    mask = np.asarray(mask, np.float32)
    weight = np.asarray(weight, np.float32)
    weight_kco = np.ascontiguousarray(
        weight.reshape(C, C, K2).transpose(1, 2, 0)).reshape(C, K2 * C)
    if "nc" not in _CACHED:
        _CACHED["nc"] = build_nc()
    nc = _CACHED["nc"]
    in_maps = [
        _prep_core(x, offset, mask, weight_kco, core) for core in range(8)
    ]
    res = run_bass_kernel_spmd(nc, in_maps, list(range(8)), trace=trace,
                               **(trace_kwargs or {}))
    out = np.empty((N, C, H, W), np.float32)
    for core in range(8):
        n, half = core // 2, core % 2
        out[n, :, half * HI:(half + 1) * HI, :] = (
            res.results[core]["out"].reshape(C, HI, W))
    return out, res


def kernel(x, offset, mask, weight):
    x = np.asarray(x, np.float32)
    offset = np.asarray(offset, np.float32)
    mask = np.asarray(mask, np.float32)
    weight = np.asarray(weight, np.float32)
    # wk[c, (k, o)] = weight[o, c, ki, kj]
    weight_kco = np.ascontiguousarray(
        weight.reshape(C, C, K2).transpose(1, 2, 0)).reshape(C, K2 * C)

    if "nc" not in _CACHED:
        _CACHED["nc"] = build_nc()
    nc = _CACHED["nc"]

    in_maps = [
        _prep_core(x, offset, mask, weight_kco, core) for core in range(8)
    ]
    res = run_bass_kernel_spmd(nc, in_maps, list(range(8)))
    out = np.empty((N, C, H, W), np.float32)
    for core in range(8):
        n, half = core // 2, core % 2
        out[n, :, half * HI:(half + 1) * HI, :] = (
            res.results[core]["out"].reshape(C, HI, W))
    return out



# revision 10
# speedup vs baseline: 2.6891x; 2.6891x over previous
"""DeformConv2d (DCNv2) Trainium2 Bass kernel, v2.

Problem: N=4, C_IN=C_OUT=64, H=W=128, 3x3 taps, stride=1, pad=1, dil=1,
modulated deformable conv (torchvision semantics).

Sharding: 8 cores; core = (image n = core//2, row-half = core%2).
Each core computes out[n, :, i0:i0+64, :] from the full image x[n].

v2 design (vs v1):
  * Row-pair interleaved fp16 image P[y, x, yc, c] in DRAM: one 512B
    gather descriptor (elem=256 fp16, step=128) fetches ALL FOUR bilinear
    corners (x0/x0+1 in-elem, y0/y0+1 via the yc interleave).  Halves both
    descriptor count (73728/core) and payload (37.7 MB/core) vs v1.
  * Offsets are host-staged in BOTH layouts (j-major for weights, 16-way
    wrapped for gather indices) — kills v1's 455 us DRAM-bounce repack.
  * fp16 combine on DVE, fp16 PE transposes/matmuls (PSUM fp32).
  * One dma_gather per (block, tap): 36 calls x 2048 descriptors.
"""
import sys
import os

_TRN_REPO = "/opt/trn_rl_repo"
if _TRN_REPO not in sys.path:
    sys.path.insert(0, _TRN_REPO)

import numpy as np

import concourse.bass as bass
import concourse.bacc as bacc
import concourse.tile as tile
import concourse.mybir as mybir
from concourse import library_config
from concourse.bass_utils import run_bass_kernel_spmd
from contextlib import ExitStack

F32 = mybir.dt.float32
F16 = mybir.dt.float16
I16 = mybir.dt.int16
ALU = mybir.AluOpType

N, C, H, W = 4, 64, 128, 128
K2 = 9
PAD = 16                    # coordinate padding on each side
PH = H + 2 * PAD            # 160
PW = W + 2 * PAD            # 160
NSLOT = PH * PW             # 25600 pixel slots; each slot = 2 rows x 64 ch
HI = 64                     # rows per core
R = 16                      # rows per block
NBLK = HI // R              # 4
CLAMP = 11.0                # |floor(offset)| clamp (pad-region safe)

_CACHED = {}


def build_nc():
    nc = bacc.Bacc(trn_type="TRN2", debug=False, num_swdge_queues=4)

    # P[y, x, yc, c] fp16: slot (y,x) holds rows y and y+1 (128 fp16 = 256B)
    xp_d = nc.dram_tensor("xp", [NSLOT * 2 * C], F16, kind="ExternalInput")
    # j-major offsets/mask for the weight path
    offj_d = nc.dram_tensor("offj", [128, 2 * K2 * HI], F32,
                            kind="ExternalInput").ap()
    maskj_d = nc.dram_tensor("maskj", [128, K2 * HI], F32,
                             kind="ExternalInput").ap()
    # wrapped offsets + index base for the gather-index path
    # free layout per partition: (two, k, i, jw) / (k, i, jw); j = 16*jw + p%16
    offw_d = nc.dram_tensor("offw", [128, 2 * K2 * HI * 8], F32,
                            kind="ExternalInput").ap()
    idxb_d = nc.dram_tensor("idxb", [128, K2 * HI * 8], F32,
                            kind="ExternalInput").ap()
    # conv weights, both parity copies: wk2[p, k*64+o] = W[o, p%64, k]
    wk2_d = nc.dram_tensor("wk2", [128, K2 * 64], F16,
                           kind="ExternalInput").ap()
    ident_d = nc.dram_tensor("ident", [128, 128], F16, kind="ExternalInput").ap()
    out_d = nc.dram_tensor("out", [64, HI * W], F32, kind="ExternalOutput").ap()

    NW = K2 * HI * 8                # 4608 free elems per partition (k, i, jw)
    # gather source: slot pairs of the interleaved image
    src_ap = bass.AP(xp_d, 0, [[2 * C, NSLOT - 1], [1, 4 * C]])

    with ExitStack() as ctx:
        tc = ctx.enter_context(tile.TileContext(nc))

        const = ctx.enter_context(tc.tile_pool(name="const", bufs=1))
        live = ctx.enter_context(tc.tile_pool(name="live", bufs=1))
        scratch_ctx = ExitStack()
        work = scratch_ctx.enter_context(tc.tile_pool(name="work", bufs=1))

        wk2 = const.tile([128, K2 * 64], F16)
        nc.sync.dma_start(wk2[:], wk2_d)
        ident = const.tile([128, 128], F16)
        nc.sync.dma_start(ident[:], ident_d)

        MAGIC = 12582912.0  # 1.5 * 2**23; rne(x) = (x+M)-M
        idxs = live.tile([128, NW], I16)

        # ---- Phase 1a: gather indices (wrapped layout, no repack) ---------
        # floor via round-to-nearest magic: floor(x) = rne(x) - (rne(x) > x).
        # dy/dx halves processed sequentially to bound SBUF (18 KB tiles).
        dyx = work.tile([128, NW], F32)
        for half in range(2):
            hctx = ExitStack()
            wh = hctx.enter_context(tc.tile_pool(name=f"wh{half}", bufs=1))
            offh = wh.tile([128, NW], F32)
            nc.sync.dma_start(offh[:], offw_d[:, half * NW:(half + 1) * NW])
            floh = wh.tile([128, NW], F32)
            nc.vector.tensor_scalar(floh[:], offh[:], MAGIC, None, ALU.add)
            nc.vector.tensor_scalar(floh[:], floh[:], MAGIC, None, ALU.subtract)
            ruph = wh.tile([128, NW], F32)
            nc.vector.tensor_tensor(ruph[:], floh[:], offh[:], ALU.is_gt)
            nc.vector.tensor_tensor(floh[:], floh[:], ruph[:], ALU.subtract)
            nc.vector.tensor_scalar(floh[:], floh[:], -CLAMP, None, ALU.max)
            nc.vector.tensor_scalar(floh[:], floh[:], CLAMP, None, ALU.min)
            if half == 0:
                nc.vector.tensor_scalar(dyx[:], floh[:], float(PW), None,
                                        ALU.mult)
            else:
                nc.vector.tensor_tensor(dyx[:], dyx[:], floh[:], ALU.add)
                idxbt = wh.tile([128, NW], F32)
                nc.sync.dma_start(idxbt[:], idxb_d)
                nc.vector.tensor_tensor(idxs[:], idxbt[:], dyx[:], ALU.add)
            hctx.close()

        # ---- Phase 1b: corner weights (j-major layout) --------------------
        offj = work.tile([128, 2 * K2 * HI], F32)
        nc.sync.dma_start(offj[:], offj_d)
        maskj = work.tile([128, K2 * HI], F32)
        nc.sync.dma_start(maskj[:], maskj_d)
        flo = work.tile([128, 2 * K2 * HI], F32)
        nc.vector.tensor_scalar(flo[:], offj[:], MAGIC, None, ALU.add)
        nc.vector.tensor_scalar(flo[:], flo[:], MAGIC, None, ALU.subtract)
        rup = work.tile([128, 2 * K2 * HI], F32)
        nc.vector.tensor_tensor(rup[:], flo[:], offj[:], ALU.is_gt)
        nc.vector.tensor_tensor(flo[:], flo[:], rup[:], ALU.subtract)
        frac = work.tile([128, 2 * K2 * HI], F32)
        nc.vector.tensor_tensor(frac[:], offj[:], flo[:], ALU.subtract)

        # offj channel layout: ch = 2k (dy), 2k+1 (dx); free = (ch, i)
        fr = frac[:].rearrange("p (k two i) -> p k two i", k=K2, two=2, i=HI)
        wy = fr[:, :, 0, :]            # [128, k, i]
        wx = fr[:, :, 1, :]
        m3 = maskj[:].rearrange("p (k i) -> p k i", k=K2, i=HI)

        # a0 = (1-wx)*m ; a1 = wx*m ; w4h[k,i,xc,yc] = a_xc * {1-wy, wy}[yc]
        a0 = work.tile([128, K2 * HI], F32)
        a0v = a0[:].rearrange("p (k i) -> p k i", k=K2, i=HI)
        nc.vector.tensor_scalar(a0v, wx, 1.0, None, ALU.subtract)
        nc.vector.tensor_scalar(a0v, a0v, -1.0, None, ALU.mult)
        nc.vector.tensor_tensor(a0v, a0v, m3, ALU.mult)
        a1 = work.tile([128, K2 * HI], F32)
        a1v = a1[:].rearrange("p (k i) -> p k i", k=K2, i=HI)
        nc.vector.tensor_tensor(a1v, wx, m3, ALU.mult)
        omy = work.tile([128, K2 * HI], F32)
        omyv = omy[:].rearrange("p (k i) -> p k i", k=K2, i=HI)
        nc.vector.tensor_scalar(omyv, wy, 1.0, None, ALU.subtract)
        nc.vector.tensor_scalar(omyv, omyv, -1.0, None, ALU.mult)

        w4h = live.tile([128, K2 * HI * 4], F16)
        w4v = w4h[:].rearrange("p (k i xc yc) -> p k i xc yc",
                               k=K2, i=HI, xc=2, yc=2)
        nc.vector.tensor_tensor(w4v[:, :, :, 0, 0], a0v, omyv, ALU.mult)
        nc.vector.tensor_tensor(w4v[:, :, :, 0, 1], a0v, wy, ALU.mult)
        nc.vector.tensor_tensor(w4v[:, :, :, 1, 0], a1v, omyv, ALU.mult)
        nc.vector.tensor_tensor(w4v[:, :, :, 1, 1], a1v, wy, ALU.mult)

        # ---- Phase 2: gather / combine / transpose / conv ----------------
        scratch_ctx.close()
        gpool = ctx.enter_context(tc.tile_pool(name="g", bufs=3))
        p4pool = ctx.enter_context(tc.tile_pool(name="p4", bufs=2))
        s2pool = ctx.enter_context(tc.tile_pool(name="s2", bufs=2))
        spool = ctx.enter_context(tc.tile_pool(name="s", bufs=2))
        stpool = ctx.enter_context(tc.tile_pool(name="st", bufs=2))
        obpool = ctx.enter_context(tc.tile_pool(name="ob", bufs=2))
        tpps = ctx.enter_context(tc.tile_pool(name="tp", bufs=2, space="PSUM"))
        outps = ctx.enter_context(tc.tile_pool(name="ops", bufs=1, space="PSUM"))

        idxs4 = idxs[:].rearrange("p (k i jw) -> p k i jw", k=K2, i=HI, jw=8)
        w4_5 = w4h[:].rearrange("p (k i xc yc) -> p k i xc yc",
                                k=K2, i=HI, xc=2, yc=2)

        for b in range(NBLK):
            # out_ps columns: (parity, h, j) — even rows 0:1024, odd 1024:2048
            out_ps = outps.tile([64, R * W], F32)
            for k in range(K2):
                g = gpool.tile([128, R * 4 * C], F16)
                gv = g[:].rearrange("p (s e) -> p s e", s=R, e=4 * C)
                RSUB = 8          # rows per dma_gather call (1024 descs)
                for sub in range(R // RSUB):
                    nidx = RSUB * 128
                    nc.gpsimd.dma_gather(
                        gv[:, sub * RSUB:(sub + 1) * RSUB, :],
                        src_ap,
                        idxs4[:, k, b * R + sub * RSUB:
                              b * R + (sub + 1) * RSUB, :],
                        nidx,
                        nidx,
                        elem_size=4 * C,
                        elem_step=2 * C,
                        queue_num=(b * K2 * 2 + k * 2 + sub) % 4,
                    )
                # weighted 4-corner combine: g layout per slot = (xc, yc, c)
                p4 = p4pool.tile([128, R * 4 * C], F16)
                wsl = w4_5[:, k, b * R:(b + 1) * R, :, :]
                w_b = bass.AP(
                    wsl.tensor, wsl.offset,
                    [wsl.ap[0], [4, R], [1, 4], [0, C]],
                )
                nc.vector.tensor_tensor(
                    p4[:].rearrange("p (i cr c) -> p i cr c", i=R, cr=4, c=C),
                    g[:].rearrange("p (i cr c) -> p i cr c", i=R, cr=4, c=C),
                    w_b, ALU.mult)
                # sum x-corners (stride 2C), then y-corners (stride C)
                s2 = s2pool.tile([128, R * 2 * C], F16)
                p4v = p4[:].rearrange("p (i xc cc) -> p i xc cc",
                                      i=R, xc=2, cc=2 * C)
                nc.vector.tensor_tensor(
                    s2[:].rearrange("p (i cc) -> p i cc", i=R, cc=2 * C),
                    p4v[:, :, 0, :], p4v[:, :, 1, :], ALU.add)
                s = spool.tile([128, R * C], F16)
                s2v = s2[:].rearrange("p (i yc c) -> p i yc c", i=R, yc=2, c=C)
                sv = s[:].rearrange("p (i c) -> p i c", i=R, c=C)
                nc.vector.tensor_tensor(
                    sv, s2v[:, :, 0, :], s2v[:, :, 1, :], ALU.add)
                # transpose row-pairs: [128 j, (2i,64c)=128] -> [(2i,c), 128 j]
                st = stpool.tile([128, (R // 2) * 128], F16)
                tp = tpps.tile([128, (R // 2) * 128], F16)
                for h in range(R // 2):
                    nc.tensor.transpose(
                        tp[:, h * 128:(h + 1) * 128],
                        s[:, h * 128:(h + 1) * 128], ident[:])
                nc.scalar.copy(st[:], tp[:])
                # conv-accumulate; st[0:64]=even rows, st[64:128]=odd rows
                for par in range(2):
                    for c2 in range(2):
                        nc.tensor.matmul(
                            out_ps[:, par * 1024 + c2 * 512:
                                   par * 1024 + (c2 + 1) * 512],
                            wk2[64 * par:64 * par + 64, k * 64:(k + 1) * 64],
                            st[64 * par:64 * par + 64, c2 * 512:(c2 + 1) * 512],
                            start=(k == 0), stop=(k == K2 - 1))
            # unshuffle (parity, h, j) -> (i, j) during PSUM drain
            ob = obpool.tile([64, R * W], F32)
            obv = ob[:].rearrange("p (h par j) -> p h par j",
                                  h=R // 2, par=2, j=W)
            opv = out_ps[:].rearrange("p (par h j) -> p par h j",
                                      par=2, h=R // 2, j=W)
            nc.scalar.copy(obv[:, :, 0, :], opv[:, 0, :, :])
            nc.scalar.copy(obv[:, :, 1, :], opv[:, 1, :, :])
            nc.sync.dma_start(out_d[:, b * R * W:(b + 1) * R * W], ob[:])

    if not nc.is_finalized():
        nc.finalize()
    return nc


def _prep_core(x, offset, mask, wk2, core):
    n, half = core // 2, core % 2
    i0 = half * HI

    # row-pair interleaved fp16 padded image P[y, x, yc, c]
    xp = np.zeros((PH, PW, C), np.float16)
    xp[PAD:PAD + H, PAD:PAD + W, :] = x[n].transpose(1, 2, 0)
    P = np.zeros((PH, PW, 2, C), np.float16)
    P[:, :, 0, :] = xp
    P[:-1, :, 1, :] = xp[1:]

    offj = np.ascontiguousarray(
        offset[n, :, i0:i0 + HI, :].transpose(2, 0, 1)).reshape(128, 2 * K2 * HI)
    maskj = np.ascontiguousarray(
        mask[n, :, i0:i0 + HI, :].transpose(2, 0, 1)).reshape(128, K2 * HI)

    # wrapped layouts: partition p holds column j = 16*jw + (p%16)
    u = np.arange(128) % 16                       # [128]
    k = np.arange(K2)
    ki, kj = k // 3, k % 3
    i = np.arange(HI)
    jw = np.arange(8)
    # offw[p, (two, k, i, jw)] : dy/dx at (i0+i, 16*jw+u)
    off5 = offset[n].reshape(K2, 2, H, W)         # [k, dy/dx, y, x]
    cols = (16 * jw[None, :] + u[:, None])        # [128, 8]
    offw = off5[:, :, i0:i0 + HI, :][:, :, :, cols]   # [k,2,i,128,8]
    offw = np.ascontiguousarray(offw.transpose(3, 1, 0, 2, 4)).reshape(128, -1)

    # idxb[p, (k, i, jw)] = slot index of the (y0, x0) corner, offsets at 0
    base = ((i0 + i[None, :, None] + ki[:, None, None] - 1 + PAD) * PW
            + jw[None, None, :] * 16 + kj[:, None, None] - 1 + PAD)  # [k,i,jw]
    idxb = base[None] + u[:, None, None, None]    # [128, k, i, jw]
    lo = idxb.min() - CLAMP * PW - CLAMP
    hi = idxb.max() + CLAMP * PW + CLAMP
    assert lo >= 0 and hi < NSLOT - 1, (lo, hi)
    idxb = idxb.reshape(128, -1).astype(np.float32)

    return {
        "xp": P.reshape(-1),
        "offj": offj,
        "maskj": maskj,
        "offw": np.ascontiguousarray(offw, np.float32),
        "idxb": idxb,
        "wk2": wk2,
        "ident": np.eye(128, dtype=np.float16),
    }


def _run(x, offset, mask, weight, trace=False, trace_kwargs=None):
    x = np.asarray(x, np.float32)
    offset = np.asarray(offset, np.float32)
    mask = np.asarray(mask, np.float32)
    weight = np.asarray(weight, np.float32)
    # wk2[p, k*64+o] = W[o, p%64, k], replicated on both partition halves
    wkco = weight.reshape(C, C, K2)               # [o, c, k]
    wk2 = np.ascontiguousarray(
        wkco.transpose(1, 2, 0)).reshape(C, K2 * C)   # [c, (k, o)]
    wk2 = np.concatenate([wk2, wk2], 0).astype(np.float16)

    if "nc" not in _CACHED:
        _CACHED["nc"] = build_nc()
    nc = _CACHED["nc"]
    in_maps = [_prep_core(x, offset, mask, wk2, core) for core in range(8)]
    if trace:
        res = run_bass_kernel_spmd(nc, in_maps, list(range(8)), trace=True,
                                   **(trace_kwargs or {}))
    else:
        res = run_bass_kernel_spmd(nc, in_maps, list(range(8)))
    out = np.empty((N, C, H, W), np.float32)
    for core in range(8):
        n, half = core // 2, core % 2
        out[n, :, half * HI:(half + 1) * HI, :] = (
            res.results[core]["out"].reshape(C, HI, W))
    return out, res


def kernel_traced(x, offset, mask, weight, trace=True, trace_kwargs=None):
    return _run(x, offset, mask, weight, trace=trace,
                trace_kwargs=trace_kwargs)


def kernel(x, offset, mask, weight):
    return _run(x, offset, mask, weight)[0]


# revision 11
# speedup vs baseline: 3.0233x; 1.1242x over previous
"""DeformConv2d (DCNv2) Trainium2 Bass kernel, v3.

Problem: N=4, C_IN=C_OUT=64, H=W=128, 3x3 taps, stride=1, pad=1, dil=1,
modulated deformable conv (torchvision semantics).

Sharding: 8 cores; core = (image n = core//2, row-half = core%2).
Each core computes out[n, :, i0:i0+64, :] from the full image x[n].

Design:
  * Row-pair interleaved fp16 image P[y, x, yc, c] in DRAM: one 512B
    gather descriptor (elem=256 fp16, step=128) fetches ALL FOUR bilinear
    corners (x0/x0+1 in-elem, y0/y0+1 via the yc interleave).
  * Offsets host-staged in BOTH layouts (j-major for weights, 16-way
    wrapped block-major for gather indices) — no on-device repack.
  * Phase 1a (gather indices) is block-pipelined: block b+1's index math
    runs on DVE while block b's gathers drain on DMA.
  * Corner weights stored duplicated (.., two=2) so the fp16 combine
    multiply walks both operands with inner step 1 (DVE 2x perf mode).
  * fp16 PE transposes of row-pairs; x-parity conv matmuls n=512.
"""
import sys
import os

_TRN_REPO = "/opt/trn_rl_repo"
if _TRN_REPO not in sys.path:
    sys.path.insert(0, _TRN_REPO)

import numpy as np

import concourse.bass as bass
import concourse.bacc as bacc
import concourse.tile as tile
import concourse.mybir as mybir
from concourse import library_config
from concourse.bass_utils import run_bass_kernel_spmd
from contextlib import ExitStack

F32 = mybir.dt.float32
F16 = mybir.dt.float16
I16 = mybir.dt.int16
ALU = mybir.AluOpType

N, C, H, W = 4, 64, 128, 128
K2 = 9
PAD = 16                    # coordinate padding on each side
PH = H + 2 * PAD            # 160
PW = W + 2 * PAD            # 160
NSLOT = PH * PW             # 25600 pixel slots; each slot = 2 rows x 64 ch
HI = 64                     # rows per core
R = 16                      # rows per block
NBLK = HI // R              # 4
CLAMP = 11.0                # |floor(offset)| clamp (pad-region safe)
NWB = 2 * K2 * R * 8        # 2304: wrapped offs per block (two, k, i, jw)
NIB = K2 * R * 8            # 1152: wrapped idx-base per block (k, i, jw)

_CACHED = {}


def build_nc():
    nc = bacc.Bacc(trn_type="TRN2", debug=False, num_swdge_queues=4)

    # P[y, x, yc, c] fp16: slot (y,x) holds rows y and y+1 (128 fp16 = 256B)
    xp_d = nc.dram_tensor("xp", [NSLOT * 2 * C], F16, kind="ExternalInput")
    # j-major offsets/mask, block-major: (b, two, k, i16) / (b, k, i16)
    offj_d = nc.dram_tensor("offj", [128, 2 * K2 * HI], F32,
                            kind="ExternalInput").ap()
    maskj_d = nc.dram_tensor("maskj", [128, K2 * HI], F32,
                             kind="ExternalInput").ap()
    # wrapped offsets + index base, block-major:
    # offw (b, two, k, i16, jw), idxb (b, k, i16, jw); j = 16*jw + p%16
    offw_d = nc.dram_tensor("offw", [128, NBLK * NWB], F32,
                            kind="ExternalInput").ap()
    idxb_d = nc.dram_tensor("idxb", [128, NBLK * NIB], F32,
                            kind="ExternalInput").ap()
    # conv weights, both parity copies: wk2[p, k*64+o] = W[o, p%64, k]
    wk2_d = nc.dram_tensor("wk2", [128, K2 * 64], F16,
                           kind="ExternalInput").ap()
    ident_d = nc.dram_tensor("ident", [128, 128], F16, kind="ExternalInput").ap()
    out_d = nc.dram_tensor("out", [64, HI * W], F32, kind="ExternalOutput").ap()

    # gather source: slot pairs of the interleaved image
    src_ap = bass.AP(xp_d, 0, [[2 * C, NSLOT - 1], [1, 4 * C]])

    MAGIC = 12582912.0  # 1.5 * 2**23; rne(x) = (x+M)-M

    with ExitStack() as ctx:
        tc = ctx.enter_context(tile.TileContext(nc))

        const = ctx.enter_context(tc.tile_pool(name="const", bufs=1))
        live = ctx.enter_context(tc.tile_pool(name="live", bufs=1))
        # per-block phase-1a scratch, double-buffered
        wpool = ctx.enter_context(tc.tile_pool(name="wp", bufs=2))

        wk2 = const.tile([128, K2 * 64], F16)
        nc.sync.dma_start(wk2[:], wk2_d)
        ident = const.tile([128, 128], F16)
        nc.sync.dma_start(ident[:], ident_d)

        idxs = live.tile([128, NBLK * NIB], I16)   # (b, k, i, jw)

        def phase1a(b):
            """gather indices for block b (wrapped layout)."""
            offh = wpool.tile([128, NWB], F32)
            nc.sync.dma_start(offh[:], offw_d[:, b * NWB:(b + 1) * NWB])
            idxbt = wpool.tile([128, NIB], F32)
            nc.sync.dma_start(idxbt[:], idxb_d[:, b * NIB:(b + 1) * NIB])
            flo = wpool.tile([128, NWB], F32)
            nc.vector.tensor_scalar(flo[:], offh[:], MAGIC, None, ALU.add)
            nc.vector.tensor_scalar(flo[:], flo[:], MAGIC, None, ALU.subtract)
            rup = wpool.tile([128, NWB], F32)
            nc.vector.tensor_tensor(rup[:], flo[:], offh[:], ALU.is_gt)
            nc.vector.tensor_tensor(flo[:], flo[:], rup[:], ALU.subtract)
            nc.vector.tensor_scalar(flo[:], flo[:], -CLAMP, None, ALU.max)
            nc.vector.tensor_scalar(flo[:], flo[:], CLAMP, None, ALU.min)
            fv = flo[:].rearrange("p (two m) -> p two m", two=2, m=NIB)
            dyx = wpool.tile([128, NIB], F32)
            nc.vector.tensor_scalar(dyx[:], fv[:, 0, :], float(PW), None,
                                    ALU.mult)
            nc.vector.tensor_tensor(dyx[:], dyx[:], fv[:, 1, :], ALU.add)
            nc.vector.tensor_tensor(idxs[:, b * NIB:(b + 1) * NIB],
                                    idxbt[:], dyx[:], ALU.add)

        phase1a(0)

        # ---- Phase 1b: corner weights, all blocks (j-major) ---------------
        p1ctx = ExitStack()
        work = p1ctx.enter_context(tc.tile_pool(name="work", bufs=1))
        offj = work.tile([128, 2 * K2 * HI], F32)
        nc.sync.dma_start(offj[:], offj_d)
        maskj = work.tile([128, K2 * HI], F32)
        nc.sync.dma_start(maskj[:], maskj_d)
        flo = work.tile([128, 2 * K2 * HI], F32)
        nc.vector.tensor_scalar(flo[:], offj[:], MAGIC, None, ALU.add)
        nc.vector.tensor_scalar(flo[:], flo[:], MAGIC, None, ALU.subtract)
        rup = work.tile([128, 2 * K2 * HI], F32)
        nc.vector.tensor_tensor(rup[:], flo[:], offj[:], ALU.is_gt)
        nc.vector.tensor_tensor(flo[:], flo[:], rup[:], ALU.subtract)
        frac = work.tile([128, 2 * K2 * HI], F32)
        nc.vector.tensor_tensor(frac[:], offj[:], flo[:], ALU.subtract)

        # offj layout: (b, two, k, i16)
        fr = frac[:].rearrange("p (b two k i) -> p b two k i",
                               b=NBLK, two=2, k=K2, i=R)
        wy = fr[:, :, 0, :, :]          # [128, b, k, i]
        wx = fr[:, :, 1, :, :]
        m3 = maskj[:].rearrange("p (b k i) -> p b k i", b=NBLK, k=K2, i=R)

        a0 = work.tile([128, K2 * HI], F32)
        a0v = a0[:].rearrange("p (b k i) -> p b k i", b=NBLK, k=K2, i=R)
        nc.vector.tensor_scalar(a0v, wx, 1.0, None, ALU.subtract)
        nc.vector.tensor_scalar(a0v, a0v, -1.0, None, ALU.mult)
        nc.vector.tensor_tensor(a0v, a0v, m3, ALU.mult)
        a1 = work.tile([128, K2 * HI], F32)
        a1v = a1[:].rearrange("p (b k i) -> p b k i", b=NBLK, k=K2, i=R)
        nc.vector.tensor_tensor(a1v, wx, m3, ALU.mult)
        omy = work.tile([128, K2 * HI], F32)
        omyv = omy[:].rearrange("p (b k i) -> p b k i", b=NBLK, k=K2, i=R)
        nc.vector.tensor_scalar(omyv, wy, 1.0, None, ALU.subtract)
        nc.vector.tensor_scalar(omyv, omyv, -1.0, None, ALU.mult)

        # w4d[(b, k, i, xc, yc, two)] fp16 — each weight duplicated (two=2)
        w4d = live.tile([128, K2 * HI * 4 * 2], F16)
        w4v = w4d[:].rearrange("p (b k i xc yc two) -> p b k i xc yc two",
                               b=NBLK, k=K2, i=R, xc=2, yc=2, two=2)
        for two in range(2):
            nc.vector.tensor_tensor(w4v[:, :, :, :, 0, 0, two], a0v, omyv,
                                    ALU.mult)
            nc.vector.tensor_tensor(w4v[:, :, :, :, 0, 1, two], a0v, wy,
                                    ALU.mult)
            nc.vector.tensor_tensor(w4v[:, :, :, :, 1, 0, two], a1v, omyv,
                                    ALU.mult)
            nc.vector.tensor_tensor(w4v[:, :, :, :, 1, 1, two], a1v, wy,
                                    ALU.mult)
        p1ctx.close()

        # ---- Phase 2: gather / combine / transpose / conv ----------------
        gpool = ctx.enter_context(tc.tile_pool(name="g", bufs=3))
        p4pool = ctx.enter_context(tc.tile_pool(name="p4", bufs=2))
        s2pool = ctx.enter_context(tc.tile_pool(name="s2", bufs=2))
        spool = ctx.enter_context(tc.tile_pool(name="s", bufs=2))
        stpool = ctx.enter_context(tc.tile_pool(name="st", bufs=2))
        obpool = ctx.enter_context(tc.tile_pool(name="ob", bufs=2))
        tpps = ctx.enter_context(tc.tile_pool(name="tp", bufs=2, space="PSUM"))
        outps = ctx.enter_context(tc.tile_pool(name="ops", bufs=1, space="PSUM"))

        idxs4 = idxs[:].rearrange("p (b k i jw) -> p b k i jw",
                                  b=NBLK, k=K2, i=R, jw=8)

        for b in range(NBLK):
            # out_ps columns: (parity, h, j) — even rows 0:1024, odd 1024:2048
            out_ps = outps.tile([64, R * W], F32)
            for k in range(K2):
                g = gpool.tile([128, R * 4 * C], F16)
                gv = g[:].rearrange("p (s e) -> p s e", s=R, e=4 * C)
                RSUB = 8          # rows per dma_gather call (1024 descs)
                for sub in range(R // RSUB):
                    nidx = RSUB * 128
                    nc.gpsimd.dma_gather(
                        gv[:, sub * RSUB:(sub + 1) * RSUB, :],
                        src_ap,
                        idxs4[:, b, k, sub * RSUB:(sub + 1) * RSUB, :],
                        nidx,
                        nidx,
                        elem_size=4 * C,
                        elem_step=2 * C,
                        queue_num=(b * K2 * 2 + k * 2 + sub) % 4,
                    )
                # weighted 4-corner combine, both operands inner step 1:
                # walk (i*xc*yc, c_hi, c_pair); weight strides (2, 0, 1)
                p4 = p4pool.tile([128, R * 4 * C], F16)
                wsl = w4v[:, b, k]
                w_b = bass.AP(
                    wsl.tensor, wsl.offset,
                    [wsl.ap[0], [2, R * 4], [0, C // 2], [1, 2]],
                )
                nc.vector.tensor_tensor(
                    p4[:].rearrange("p (icr chi c2) -> p icr chi c2",
                                    icr=R * 4, chi=C // 2, c2=2),
                    g[:].rearrange("p (icr chi c2) -> p icr chi c2",
                                   icr=R * 4, chi=C // 2, c2=2),
                    w_b, ALU.mult)
                if k == 0 and b + 1 < NBLK:
                    phase1a(b + 1)
                # sum x-corners (stride 2C), then y-corners (stride C)
                s2 = s2pool.tile([128, R * 2 * C], F16)
                p4v = p4[:].rearrange("p (i xc cc) -> p i xc cc",
                                      i=R, xc=2, cc=2 * C)
                nc.vector.tensor_tensor(
                    s2[:].rearrange("p (i cc) -> p i cc", i=R, cc=2 * C),
                    p4v[:, :, 0, :], p4v[:, :, 1, :], ALU.add)
                s = spool.tile([128, R * C], F16)
                s2v = s2[:].rearrange("p (i yc c) -> p i yc c", i=R, yc=2, c=C)
                sv = s[:].rearrange("p (i c) -> p i c", i=R, c=C)
                nc.vector.tensor_tensor(
                    sv, s2v[:, :, 0, :], s2v[:, :, 1, :], ALU.add)
                # transpose row-pairs: [128 j, (2i,64c)=128] -> [(2i,c), 128 j]
                st = stpool.tile([128, (R // 2) * 128], F16)
                tp = tpps.tile([128, (R // 2) * 128], F16)
                for h in range(R // 2):
                    nc.tensor.transpose(
                        tp[:, h * 128:(h + 1) * 128],
                        s[:, h * 128:(h + 1) * 128], ident[:])
                nc.scalar.copy(st[:], tp[:])
                # conv-accumulate; st[0:64]=even rows, st[64:128]=odd rows
                for par in range(2):
                    for c2 in range(2):
                        nc.tensor.matmul(
                            out_ps[:, par * 1024 + c2 * 512:
                                   par * 1024 + (c2 + 1) * 512],
                            wk2[64 * par:64 * par + 64, k * 64:(k + 1) * 64],
                            st[64 * par:64 * par + 64, c2 * 512:(c2 + 1) * 512],
                            start=(k == 0), stop=(k == K2 - 1))
            # unshuffle (parity, h, j) -> (i, j) during PSUM drain
            ob = obpool.tile([64, R * W], F32)
            obv = ob[:].rearrange("p (h par j) -> p h par j",
                                  h=R // 2, par=2, j=W)
            opv = out_ps[:].rearrange("p (par h j) -> p par h j",
                                      par=2, h=R // 2, j=W)
            nc.scalar.copy(obv[:, :, 0, :], opv[:, 0, :, :])
            nc.scalar.copy(obv[:, :, 1, :], opv[:, 1, :, :])
            nc.sync.dma_start(out_d[:, b * R * W:(b + 1) * R * W], ob[:])

    if not nc.is_finalized():
        nc.finalize()
    return nc


def _prep_core(x, offset, mask, wk2, core):
    n, half = core // 2, core % 2
    i0 = half * HI

    # row-pair interleaved fp16 padded image P[y, x, yc, c]
    xp = np.zeros((PH, PW, C), np.float16)
    xp[PAD:PAD + H, PAD:PAD + W, :] = x[n].transpose(1, 2, 0)
    P = np.zeros((PH, PW, 2, C), np.float16)
    P[:, :, 0, :] = xp
    P[:-1, :, 1, :] = xp[1:]

    # j-major, block-major: offj (b, two, k, i16); maskj (b, k, i16)
    offj = offset[n, :, i0:i0 + HI, :].transpose(2, 0, 1)   # [j, 2K2, i]
    offj = offj.reshape(128, K2, 2, NBLK, R)                # ch = (k, two)
    offj = np.ascontiguousarray(
        offj.transpose(0, 3, 2, 1, 4)).reshape(128, 2 * K2 * HI)
    maskj = mask[n, :, i0:i0 + HI, :].transpose(2, 0, 1)    # [j, k, i]
    maskj = maskj.reshape(128, K2, NBLK, R)
    maskj = np.ascontiguousarray(
        maskj.transpose(0, 2, 1, 3)).reshape(128, K2 * HI)

    # wrapped layouts: partition p holds column j = 16*jw + (p%16)
    u = np.arange(128) % 16                       # [128]
    k = np.arange(K2)
    ki, kj = k // 3, k % 3
    i = np.arange(R)
    jw = np.arange(8)
    # offw[p, (b, two, k, i16, jw)]
    off5 = offset[n].reshape(K2, 2, H, W)         # [k, dy/dx, y, x]
    cols = (16 * jw[None, :] + u[:, None])        # [128, 8]
    offw = off5[:, :, i0:i0 + HI, :][:, :, :, cols]   # [k,2,i64,128,8]
    offw = offw.reshape(K2, 2, NBLK, R, 128, 8)
    offw = np.ascontiguousarray(
        offw.transpose(4, 2, 1, 0, 3, 5)).reshape(128, -1)

    # idxb[p, (b, k, i16, jw)] = slot index of the (y0, x0) corner
    b4 = np.arange(NBLK)
    base = ((i0 + b4[:, None, None, None] * R + i[None, None, :, None]
             + ki[None, :, None, None] - 1 + PAD) * PW
            + jw[None, None, None, :] * 16
            + kj[None, :, None, None] - 1 + PAD)          # [b, k, i, jw]
    idxb = base[None] + u[:, None, None, None, None]      # [128, b, k, i, jw]
    lo = idxb.min() - CLAMP * PW - CLAMP
    hi = idxb.max() + CLAMP * PW + CLAMP
    assert lo >= 0 and hi < NSLOT - 1, (lo, hi)
    idxb = idxb.reshape(128, -1).astype(np.float32)

    return {
        "xp": P.reshape(-1),
        "offj": offj,
        "maskj": maskj,
        "offw": np.ascontiguousarray(offw, np.float32),
        "idxb": idxb,
        "wk2": wk2,
        "ident": np.eye(128, dtype=np.float16),
    }


def _run(x, offset, mask, weight, trace=False, trace_kwargs=None):
    x = np.asarray(x, np.float32)
    offset = np.asarray(offset, np.float32)
    mask = np.asarray(mask, np.float32)
    weight = np.asarray(weight, np.float32)
    # wk2[p, k*64+o] = W[o, p%64, k], replicated on both partition halves
    wkco = weight.reshape(C, C, K2)               # [o, c, k]
    wk2 = np.ascontiguousarray(
        wkco.transpose(1, 2, 0)).reshape(C, K2 * C)   # [c, (k, o)]
    wk2 = np.concatenate([wk2, wk2], 0).astype(np.float16)

    if "nc" not in _CACHED:
        _CACHED["nc"] = build_nc()
    nc = _CACHED["nc"]
    in_maps = [_prep_core(x, offset, mask, wk2, core) for core in range(8)]
    if trace:
        res = run_bass_kernel_spmd(nc, in_maps, list(range(8)), trace=True,
                                   **(trace_kwargs or {}))
    else:
        res = run_bass_kernel_spmd(nc, in_maps, list(range(8)))
    out = np.empty((N, C, H, W), np.float32)
    for core in range(8):
        n, half = core // 2, core % 2
        out[n, :, half * HI:(half + 1) * HI, :] = (
            res.results[core]["out"].reshape(C, HI, W))
    return out, res


def kernel_traced(x, offset, mask, weight, trace=True, trace_kwargs=None):
    return _run(x, offset, mask, weight, trace=trace,
                trace_kwargs=trace_kwargs)


def kernel(x, offset, mask, weight):
    return _run(x, offset, mask, weight)[0]


# revision 18
# speedup vs baseline: 4.1012x; 1.3565x over previous
"""DeformConv2d (DCNv2) Trainium2 Bass kernel, v3.

Problem: N=4, C_IN=C_OUT=64, H=W=128, 3x3 taps, stride=1, pad=1, dil=1,
modulated deformable conv (torchvision semantics).

Sharding: 8 cores; core = (image n = core//2, row-half = core%2).
Each core computes out[n, :, i0:i0+64, :] from the full image x[n].

Design:
  * Row-pair interleaved fp16 image P[y, x, yc, c] in DRAM: one 512B
    gather descriptor (elem=256 fp16, step=128) fetches ALL FOUR bilinear
    corners (x0/x0+1 in-elem, y0/y0+1 via the yc interleave).
  * Offsets host-staged in BOTH layouts (j-major for weights, 16-way
    wrapped block-major for gather indices) — no on-device repack.
  * Phase 1a (gather indices) is block-pipelined: block b+1's index math
    runs on DVE while block b's gathers drain on DMA.
  * Corner weights stored duplicated (.., two=2) so the fp16 combine
    multiply walks both operands with inner step 1 (DVE 2x perf mode).
  * fp16 PE transposes of row-pairs; x-parity conv matmuls n=512.
"""
import sys
import os

_TRN_REPO = "/opt/trn_rl_repo"
if _TRN_REPO not in sys.path:
    sys.path.insert(0, _TRN_REPO)

import numpy as np

import concourse.bass as bass
import concourse.bacc as bacc
import concourse.tile as tile
import concourse.mybir as mybir
from concourse import library_config
from concourse.bass_utils import run_bass_kernel_spmd
from contextlib import ExitStack

F32 = mybir.dt.float32
F16 = mybir.dt.float16
I16 = mybir.dt.int16
ALU = mybir.AluOpType

N, C, H, W = 4, 64, 128, 128
K2 = 9
PAD = 16                    # coordinate padding on each side
PH = H + 2 * PAD            # 160
PW = W + 2 * PAD            # 160
NSLOT = PH * PW             # 25600 pixel slots; each slot = 2 rows x 64 ch
HI = 64                     # rows per core
R = 16                      # rows per block
NBLK = HI // R              # 4
CLAMP = 11.0                # |floor(offset)| clamp (pad-region safe)
NWB = 2 * K2 * R * 8        # 2304: wrapped offs per block (two, k, i, jw)
NIB = K2 * R * 8            # 1152: wrapped idx-base per block (k, i, jw)

_CACHED = {}


def build_nc():
    nc = bacc.Bacc(trn_type="TRN2", debug=False, num_swdge_queues=4)

    # P[y, x, yc, c] fp16: slot (y,x) holds rows y and y+1 (128 fp16 = 256B)
    xp_d = nc.dram_tensor("xp", [NSLOT * 2 * C], F16, kind="ExternalInput")
    # j-major offsets/mask, block-major: (b, two, k, i16) / (b, k, i16)
    offj_d = nc.dram_tensor("offj", [128, 2 * K2 * HI], F32,
                            kind="ExternalInput").ap()
    maskj_d = nc.dram_tensor("maskj", [128, K2 * HI], F32,
                             kind="ExternalInput").ap()
    # wrapped offsets + index base, block-major:
    # offw (b, two, k, i16, jw), idxb (b, k, i16, jw); j = 16*jw + p%16
    offw_d = nc.dram_tensor("offw", [128, NBLK * NWB], F32,
                            kind="ExternalInput").ap()
    idxb_d = nc.dram_tensor("idxb", [128, NBLK * NIB], F32,
                            kind="ExternalInput").ap()
    # conv weights, both parity copies: wk2[p, k*64+o] = W[o, p%64, k]
    wk2_d = nc.dram_tensor("wk2", [128, K2 * 64], F16,
                           kind="ExternalInput").ap()
    ident_d = nc.dram_tensor("ident", [128, 128], F16, kind="ExternalInput").ap()
    out_d = nc.dram_tensor("out", [64, HI * W], F32, kind="ExternalOutput").ap()

    # gather source: slot pairs of the interleaved image
    src_ap = bass.AP(xp_d, 0, [[2 * C, NSLOT - 1], [1, 4 * C]])

    MAGIC = 12582912.0  # 1.5 * 2**23; rne(x) = (x+M)-M

    with ExitStack() as ctx:
        tc = ctx.enter_context(tile.TileContext(nc))

        const = ctx.enter_context(tc.tile_pool(name="const", bufs=1))
        live = ctx.enter_context(tc.tile_pool(name="live", bufs=1))
        # per-block phase-1a scratch, double-buffered
        wpool = ctx.enter_context(tc.tile_pool(name="wp", bufs=2))

        wk2 = const.tile([128, K2 * 64], F16)
        nc.sync.dma_start(wk2[:], wk2_d)
        ident = const.tile([128, 128], F16)
        nc.sync.dma_start(ident[:], ident_d)

        idxs = live.tile([128, NBLK * NIB], I16)   # (b, k, i, jw)
        COPY = mybir.ActivationFunctionType.Copy

        def phase1a(b):
            """gather indices for block b (wrapped layout).

            floor(x) = rne(x - 0.5) — bilinear-safe (the frac computed with
            the same floor compensates at ties/integers; pad margin covers
            the off-by-one at exact odd integers).  Runs on ACT to keep DVE
            free; host pre-clips offsets to +-CLAMP.
            """
            offh = wpool.tile([128, NWB], F32)
            nc.sync.dma_start(offh[:], offw_d[:, b * NWB:(b + 1) * NWB])
            idxbt = wpool.tile([128, NIB], F32)
            nc.sync.dma_start(idxbt[:], idxb_d[:, b * NIB:(b + 1) * NIB])
            flo = wpool.tile([128, NWB], F32)
            nc.scalar.activation(flo[:], offh[:], COPY, bias=-0.5)
            nc.scalar.activation(flo[:], flo[:], COPY, bias=MAGIC)
            nc.scalar.activation(flo[:], flo[:], COPY, bias=-MAGIC)
            fv = flo[:].rearrange("p (two m) -> p two m", two=2, m=NIB)
            dyx = wpool.tile([128, NIB], F32)
            nc.scalar.activation(dyx[:], fv[:, 0, :], COPY, scale=float(PW))
            nc.vector.tensor_tensor(dyx[:], dyx[:], fv[:, 1, :], ALU.add)
            nc.vector.tensor_tensor(idxs[:, b * NIB:(b + 1) * NIB],
                                    idxbt[:], dyx[:], ALU.add)

        phase1a(0)

        # ---- Phase 1b: corner weights, all blocks (j-major) ---------------
        p1ctx = ExitStack()
        work = p1ctx.enter_context(tc.tile_pool(name="work", bufs=1))
        offj = work.tile([128, 2 * K2 * HI], F32)
        nc.sync.dma_start(offj[:], offj_d)
        maskj = work.tile([128, K2 * HI], F32)
        nc.sync.dma_start(maskj[:], maskj_d)
        flo = work.tile([128, 2 * K2 * HI], F32)
        nc.scalar.activation(flo[:], offj[:], COPY, bias=-0.5)
        nc.scalar.activation(flo[:], flo[:], COPY, bias=MAGIC)
        nc.scalar.activation(flo[:], flo[:], COPY, bias=-MAGIC)
        frac = work.tile([128, 2 * K2 * HI], F32)
        nc.vector.tensor_tensor(frac[:], offj[:], flo[:], ALU.subtract)

        # offj layout: (b, two, k, i16)
        fr = frac[:].rearrange("p (b two k i) -> p b two k i",
                               b=NBLK, two=2, k=K2, i=R)
        wy = fr[:, :, 0, :, :]          # [128, b, k, i]
        wx = fr[:, :, 1, :, :]
        m3 = maskj[:].rearrange("p (b k i) -> p b k i", b=NBLK, k=K2, i=R)

        a0 = work.tile([128, K2 * HI], F32)
        a0v = a0[:].rearrange("p (b k i) -> p b k i", b=NBLK, k=K2, i=R)
        nc.scalar.activation(a0v, wx, COPY, bias=1.0, scale=-1.0)
        nc.vector.tensor_tensor(a0v, a0v, m3, ALU.mult)
        a1 = work.tile([128, K2 * HI], F32)
        a1v = a1[:].rearrange("p (b k i) -> p b k i", b=NBLK, k=K2, i=R)
        nc.vector.tensor_tensor(a1v, wx, m3, ALU.mult)
        omy = work.tile([128, K2 * HI], F32)
        omyv = omy[:].rearrange("p (b k i) -> p b k i", b=NBLK, k=K2, i=R)
        nc.scalar.activation(omyv, wy, COPY, bias=1.0, scale=-1.0)

        # w4d[(b, k, i, xc, yc, two)] fp16 — each weight duplicated (two=2);
        # each mult writes both dups (dst pairs contiguous, srcs 0-stride).
        w4d = live.tile([128, K2 * HI * 4 * 2], F16)
        w4v = w4d[:].rearrange("p (b k i xc yc two) -> p b k i xc yc two",
                               b=NBLK, k=K2, i=R, xc=2, yc=2, two=2)

        # all operands as [p, b(4), ki(144), two(2)]
        KI = K2 * R

        def dup_flat(t):   # contiguous [128, 576] tile
            ap = t[:]
            return bass.AP(ap.tensor, ap.offset,
                           [ap.ap[0], [KI, NBLK], [1, KI], [0, 2]])

        # wy view (b,k,i) strides (288,16,1) within frac's (b,two,k,i)
        wy_dup = bass.AP(wy.tensor, wy.offset,
                         [wy.ap[0], [2 * KI, NBLK], [1, KI], [0, 2]])

        for xc, yc, asrc in ((0, 0, a0), (0, 1, a0), (1, 0, a1), (1, 1, a1)):
            dst = w4v[:, :, :, :, xc, yc, :]
            dst = bass.AP(dst.tensor, dst.offset,
                          [dst.ap[0], [8 * KI, NBLK], [8, KI], [1, 2]])
            ysrc = dup_flat(omy) if yc == 0 else wy_dup
            nc.vector.tensor_tensor(dst, dup_flat(asrc), ysrc, ALU.mult)
        p1ctx.close()

        # ---- Phase 2: gather / combine / transpose / conv ----------------
        gpool = ctx.enter_context(tc.tile_pool(name="g", bufs=4))
        p4pool = ctx.enter_context(tc.tile_pool(name="p4", bufs=2))
        s2pool = ctx.enter_context(tc.tile_pool(name="s2", bufs=2))
        spool = ctx.enter_context(tc.tile_pool(name="s", bufs=2))
        stpool = ctx.enter_context(tc.tile_pool(name="st", bufs=2))
        obpool = ctx.enter_context(tc.tile_pool(name="ob", bufs=2))
        tpps = ctx.enter_context(tc.tile_pool(name="tp", bufs=2, space="PSUM"))
        outps = ctx.enter_context(tc.tile_pool(name="ops", bufs=1, space="PSUM"))

        idxs4 = idxs[:].rearrange("p (b k i jw) -> p b k i jw",
                                  b=NBLK, k=K2, i=R, jw=8)

        for b in range(NBLK):
            # out_ps columns: (parity, h, j) — even rows 0:1024, odd 1024:2048
            out_ps = outps.tile([64, R * W], F32)
            for k in range(K2):
                g = gpool.tile([128, R * 4 * C], F16)
                gv = g[:].rearrange("p (s e) -> p s e", s=R, e=4 * C)
                RSUB = 8          # rows per dma_gather call (1024 descs)
                for sub in range(R // RSUB):
                    nidx = RSUB * 128
                    nc.gpsimd.dma_gather(
                        gv[:, sub * RSUB:(sub + 1) * RSUB, :],
                        src_ap,
                        idxs4[:, b, k, sub * RSUB:(sub + 1) * RSUB, :],
                        nidx,
                        nidx,
                        elem_size=4 * C,
                        elem_step=2 * C,
                        queue_num=(b * K2 * 2 + k * 2 + sub) % 4,
                    )
                # weighted 4-corner combine, both operands inner step 1:
                # walk (i*xc*yc, c_hi, c_pair); weight strides (2, 0, 1)
                p4 = p4pool.tile([128, R * 4 * C], F16)
                wsl = w4v[:, b, k]
                w_b = bass.AP(
                    wsl.tensor, wsl.offset,
                    [wsl.ap[0], [2, R * 4], [0, C // 2], [1, 2]],
                )
                nc.vector.tensor_tensor(
                    p4[:].rearrange("p (icr chi c2) -> p icr chi c2",
                                    icr=R * 4, chi=C // 2, c2=2),
                    g[:].rearrange("p (icr chi c2) -> p icr chi c2",
                                   icr=R * 4, chi=C // 2, c2=2),
                    w_b, ALU.mult)
                if k == 0 and b + 1 < NBLK:
                    phase1a(b + 1)
                # sum x-corners (stride 2C), then y-corners (stride C)
                s2 = s2pool.tile([128, R * 2 * C], F16)
                p4v = p4[:].rearrange("p (i xc cc) -> p i xc cc",
                                      i=R, xc=2, cc=2 * C)
                nc.vector.tensor_tensor(
                    s2[:].rearrange("p (i cc) -> p i cc", i=R, cc=2 * C),
                    p4v[:, :, 0, :], p4v[:, :, 1, :], ALU.add)
                s = spool.tile([128, R * C], F16)
                s2v = s2[:].rearrange("p (i yc c) -> p i yc c", i=R, yc=2, c=C)
                sv = s[:].rearrange("p (i c) -> p i c", i=R, c=C)
                nc.vector.tensor_tensor(
                    sv, s2v[:, :, 0, :], s2v[:, :, 1, :], ALU.add)
                # transpose row-pairs: [128 j, (2i,64c)=128] -> [(2i,c), 128 j]
                st = stpool.tile([128, (R // 2) * 128], F16)
                tp = tpps.tile([128, (R // 2) * 128], F16)
                for h in range(R // 2):
                    nc.tensor.transpose(
                        tp[:, h * 128:(h + 1) * 128],
                        s[:, h * 128:(h + 1) * 128], ident[:])
                nc.scalar.copy(st[:], tp[:])
                # conv-accumulate; st[0:64]=even rows, st[64:128]=odd rows
                for par in range(2):
                    for c2 in range(2):
                        nc.tensor.matmul(
                            out_ps[:, par * 1024 + c2 * 512:
                                   par * 1024 + (c2 + 1) * 512],
                            wk2[64 * par:64 * par + 64, k * 64:(k + 1) * 64],
                            st[64 * par:64 * par + 64, c2 * 512:(c2 + 1) * 512],
                            start=(k == 0), stop=(k == K2 - 1))
            # unshuffle (parity, h, j) -> (i, j) during PSUM drain
            ob = obpool.tile([64, R * W], F32)
            obv = ob[:].rearrange("p (h par j) -> p h par j",
                                  h=R // 2, par=2, j=W)
            opv = out_ps[:].rearrange("p (par h j) -> p par h j",
                                      par=2, h=R // 2, j=W)
            nc.scalar.copy(obv[:, :, 0, :], opv[:, 0, :, :])
            nc.scalar.copy(obv[:, :, 1, :], opv[:, 1, :, :])
            nc.sync.dma_start(out_d[:, b * R * W:(b + 1) * R * W], ob[:])

    if not nc.is_finalized():
        nc.finalize()
    return nc


def _prep_core(x, offset, mask, wk2, core):
    n, half = core // 2, core % 2
    i0 = half * HI
    # clamp on host (device floor has no clamp); keeps gather slots in-pad
    offset = np.clip(offset, -CLAMP, CLAMP)

    # row-pair interleaved fp16 padded image P[y, x, yc, c]
    xp = np.zeros((PH, PW, C), np.float16)
    xp[PAD:PAD + H, PAD:PAD + W, :] = x[n].transpose(1, 2, 0)
    P = np.zeros((PH, PW, 2, C), np.float16)
    P[:, :, 0, :] = xp
    P[:-1, :, 1, :] = xp[1:]

    # j-major, block-major: offj (b, two, k, i16); maskj (b, k, i16)
    offj = offset[n, :, i0:i0 + HI, :].transpose(2, 0, 1)   # [j, 2K2, i]
    offj = offj.reshape(128, K2, 2, NBLK, R)                # ch = (k, two)
    offj = np.ascontiguousarray(
        offj.transpose(0, 3, 2, 1, 4)).reshape(128, 2 * K2 * HI)
    maskj = mask[n, :, i0:i0 + HI, :].transpose(2, 0, 1)    # [j, k, i]
    maskj = maskj.reshape(128, K2, NBLK, R)
    maskj = np.ascontiguousarray(
        maskj.transpose(0, 2, 1, 3)).reshape(128, K2 * HI)

    # wrapped layouts: partition p holds column j = 16*jw + (p%16)
    u = np.arange(128) % 16                       # [128]
    k = np.arange(K2)
    ki, kj = k // 3, k % 3
    i = np.arange(R)
    jw = np.arange(8)
    # offw[p, (b, two, k, i16, jw)]
    off5 = offset[n].reshape(K2, 2, H, W)         # [k, dy/dx, y, x]
    cols = (16 * jw[None, :] + u[:, None])        # [128, 8]
    offw = off5[:, :, i0:i0 + HI, :][:, :, :, cols]   # [k,2,i64,128,8]
    offw = offw.reshape(K2, 2, NBLK, R, 128, 8)
    offw = np.ascontiguousarray(
        offw.transpose(4, 2, 1, 0, 3, 5)).reshape(128, -1)

    # idxb[p, (b, k, i16, jw)] = slot index of the (y0, x0) corner
    b4 = np.arange(NBLK)
    base = ((i0 + b4[:, None, None, None] * R + i[None, None, :, None]
             + ki[None, :, None, None] - 1 + PAD) * PW
            + jw[None, None, None, :] * 16
            + kj[None, :, None, None] - 1 + PAD)          # [b, k, i, jw]
    idxb = base[None] + u[:, None, None, None, None]      # [128, b, k, i, jw]
    lo = idxb.min() - CLAMP * PW - CLAMP
    hi = idxb.max() + CLAMP * PW + CLAMP
    assert lo >= 0 and hi < NSLOT - 1, (lo, hi)
    idxb = idxb.reshape(128, -1).astype(np.float32)

    return {
        "xp": P.reshape(-1),
        "offj": offj,
        "maskj": maskj,
        "offw": np.ascontiguousarray(offw, np.float32),
        "idxb": idxb,
        "wk2": wk2,
        "ident": np.eye(128, dtype=np.float16),
    }


def _run(x, offset, mask, weight, trace=False, trace_kwargs=None):
    x = np.asarray(x, np.float32)
    offset = np.asarray(offset, np.float32)
    mask = np.asarray(mask, np.float32)
    weight = np.asarray(weight, np.float32)
    # wk2[p, k*64+o] = W[o, p%64, k], replicated on both partition halves
    wkco = weight.reshape(C, C, K2)               # [o, c, k]
    wk2 = np.ascontiguousarray(
        wkco.transpose(1, 2, 0)).reshape(C, K2 * C)   # [c, (k, o)]
    wk2 = np.concatenate([wk2, wk2], 0).astype(np.float16)

    if "nc" not in _CACHED:
        _CACHED["nc"] = build_nc()
    nc = _CACHED["nc"]
    in_maps = [_prep_core(x, offset, mask, wk2, core) for core in range(8)]
    if trace:
        res = run_bass_kernel_spmd(nc, in_maps, list(range(8)), trace=True,
                                   **(trace_kwargs or {}))
    else:
        res = run_bass_kernel_spmd(nc, in_maps, list(range(8)))
    out = np.empty((N, C, H, W), np.float32)
    for core in range(8):
        n, half = core // 2, core % 2
        out[n, :, half * HI:(half + 1) * HI, :] = (
            res.results[core]["out"].reshape(C, HI, W))
    return out, res


def kernel_traced(x, offset, mask, weight, trace=True, trace_kwargs=None):
    return _run(x, offset, mask, weight, trace=trace,
                trace_kwargs=trace_kwargs)


def kernel(x, offset, mask, weight):
    return _run(x, offset, mask, weight)[0]


# revision 23
# speedup vs baseline: 4.5335x; 1.1054x over previous
"""DeformConv2d (DCNv2) Trainium2 Bass kernel, v3.

Problem: N=4, C_IN=C_OUT=64, H=W=128, 3x3 taps, stride=1, pad=1, dil=1,
modulated deformable conv (torchvision semantics).

Sharding: 8 cores; core = (image n = core//2, row-half = core%2).
Each core computes out[n, :, i0:i0+64, :] from the full image x[n].

Design:
  * Row-pair interleaved fp16 image P[y, x, yc, c] in DRAM: one 512B
    gather descriptor (elem=256 fp16, step=128) fetches ALL FOUR bilinear
    corners (x0/x0+1 in-elem, y0/y0+1 via the yc interleave).
  * Offsets host-staged in BOTH layouts (j-major for weights, 16-way
    wrapped block-major for gather indices) — no on-device repack.
  * Phase 1a (gather indices) is block-pipelined: block b+1's index math
    runs on DVE while block b's gathers drain on DMA.
  * Corner weights stored duplicated (.., two=2) so the fp16 combine
    multiply walks both operands with inner step 1 (DVE 2x perf mode).
  * fp16 PE transposes of row-pairs; x-parity conv matmuls n=512.
"""
import sys
import os

_TRN_REPO = "/opt/trn_rl_repo"
if _TRN_REPO not in sys.path:
    sys.path.insert(0, _TRN_REPO)

import numpy as np

import concourse.bass as bass
import concourse.bacc as bacc
import concourse.tile as tile
import concourse.mybir as mybir
from concourse import library_config
from concourse.bass_utils import run_bass_kernel_spmd
from contextlib import ExitStack

F32 = mybir.dt.float32
F16 = mybir.dt.float16
I16 = mybir.dt.int16
ALU = mybir.AluOpType

N, C, H, W = 4, 64, 128, 128
K2 = 9
PAD = 16                    # coordinate padding on each side
PH = H + 2 * PAD            # 160
PW = W + 2 * PAD            # 160
NSLOT = PH * PW             # 25600 pixel slots; each slot = 2 rows x 64 ch
HI = 64                     # rows per core
R = 16                      # rows per block
NBLK = HI // R              # 4
CLAMP = 11.0                # |floor(offset)| clamp (pad-region safe)
NWB = 2 * K2 * R * 8        # 2304: wrapped offs per block (two, k, i, jw)
NIB = K2 * R * 8            # 1152: wrapped idx-base per block (k, i, jw)

_CACHED = {}


def build_nc():
    nc = bacc.Bacc(trn_type="TRN2", debug=False, num_swdge_queues=4)

    # P[y, x, yc, c] fp16: slot (y,x) holds rows y and y+1 (128 fp16 = 256B)
    xp_d = nc.dram_tensor("xp", [NSLOT * 2 * C], F16, kind="ExternalInput")
    # j-major offsets/mask, block-major: (b, two, k, i16) / (b, k, i16)
    offj_d = nc.dram_tensor("offj", [128, 2 * K2 * HI], F32,
                            kind="ExternalInput").ap()
    maskj_d = nc.dram_tensor("maskj", [128, K2 * HI], F32,
                             kind="ExternalInput").ap()
    # wrapped offsets + index base, block-major:
    # offw (b, two, k, i16, jw), idxb (b, k, i16, jw); j = 16*jw + p%16
    offw_d = nc.dram_tensor("offw", [128, NBLK * NWB], F32,
                            kind="ExternalInput").ap()
    idxb_d = nc.dram_tensor("idxb", [128, NBLK * NIB], F32,
                            kind="ExternalInput").ap()
    # conv weights, both parity copies: wk2[p, k*64+o] = W[o, p%64, k]
    wk2_d = nc.dram_tensor("wk2", [128, K2 * 64], F16,
                           kind="ExternalInput").ap()
    ident_d = nc.dram_tensor("ident", [128, 128], F16, kind="ExternalInput").ap()
    out_d = nc.dram_tensor("out", [64, HI * W], F32, kind="ExternalOutput").ap()

    # gather source: slot pairs of the interleaved image
    src_ap = bass.AP(xp_d, 0, [[2 * C, NSLOT - 1], [1, 4 * C]])

    MAGIC = 12582912.0  # 1.5 * 2**23; rne(x) = (x+M)-M

    with ExitStack() as ctx:
        tc = ctx.enter_context(tile.TileContext(nc))

        const = ctx.enter_context(tc.tile_pool(name="const", bufs=1))
        live = ctx.enter_context(tc.tile_pool(name="live", bufs=1))
        # per-block phase-1a scratch, double-buffered
        wpool = ctx.enter_context(tc.tile_pool(name="wp", bufs=2))

        wk2 = const.tile([128, K2 * 64], F16)
        nc.sync.dma_start(wk2[:], wk2_d)
        ident = const.tile([128, 128], F16)
        nc.sync.dma_start(ident[:], ident_d)

        idxs = live.tile([128, NBLK * NIB], I16)   # (b, k, i, jw)
        COPY = mybir.ActivationFunctionType.Copy

        def phase1a(b):
            """gather indices for block b (wrapped layout).

            floor(x) = rne(x - 0.5) — bilinear-safe (the frac computed with
            the same floor compensates at ties/integers; pad margin covers
            the off-by-one at exact odd integers).  Runs on ACT to keep DVE
            free; host pre-clips offsets to +-CLAMP.
            """
            offh = wpool.tile([128, NWB], F32)
            nc.sync.dma_start(offh[:], offw_d[:, b * NWB:(b + 1) * NWB])
            idxbt = wpool.tile([128, NIB], F32)
            nc.sync.dma_start(idxbt[:], idxb_d[:, b * NIB:(b + 1) * NIB])
            # floor in-place on offh
            nc.scalar.activation(offh[:], offh[:], COPY, bias=-0.5)
            nc.scalar.activation(offh[:], offh[:], COPY, bias=MAGIC)
            nc.scalar.activation(offh[:], offh[:], COPY, bias=-MAGIC)
            fv = offh[:].rearrange("p (two m) -> p two m", two=2, m=NIB)
            dyx = wpool.tile([128, NIB], F32)
            nc.scalar.activation(dyx[:], fv[:, 0, :], COPY, scale=float(PW))
            nc.vector.tensor_tensor(dyx[:], dyx[:], fv[:, 1, :], ALU.add)
            nc.vector.tensor_tensor(idxs[:, b * NIB:(b + 1) * NIB],
                                    idxbt[:], dyx[:], ALU.add)

        phase1a(0)

        # ---- Phase 1b: corner weights, all blocks (j-major) ---------------
        # NOTE: this pool stays OPEN for the whole kernel.  Closing it makes
        # the phase-2 pools reuse its SBUF, which inserts a pool-reuse
        # barrier on the GpSimd queue — the first gather then waits for all
        # of phase 1b (measured 17 us stall).  Emission is deferred until
        # after the first gather calls so phase-1a(0)'s ACT chain runs first.
        work = ctx.enter_context(tc.tile_pool(name="work", bufs=1))
        w4d = live.tile([128, K2 * HI * 4 * 2], F16)
        w4v = w4d[:].rearrange("p (b k i xc yc two) -> p b k i xc yc two",
                               b=NBLK, k=K2, i=R, xc=2, yc=2, two=2)

        def phase1b():
            offj = work.tile([128, 2 * K2 * HI], F32)
            nc.sync.dma_start(offj[:], offj_d)
            maskj = work.tile([128, K2 * HI], F32)
            nc.sync.dma_start(maskj[:], maskj_d)
            flo = work.tile([128, 2 * K2 * HI], F32)
            nc.scalar.activation(flo[:], offj[:], COPY, bias=-0.5)
            nc.scalar.activation(flo[:], flo[:], COPY, bias=MAGIC)
            nc.scalar.activation(flo[:], flo[:], COPY, bias=-MAGIC)
            frac = work.tile([128, 2 * K2 * HI], F32)
            nc.vector.tensor_tensor(frac[:], offj[:], flo[:], ALU.subtract)

            # offj layout: (b, two, k, i16)
            fr = frac[:].rearrange("p (b two k i) -> p b two k i",
                                   b=NBLK, two=2, k=K2, i=R)
            wy = fr[:, :, 0, :, :]          # [128, b, k, i]
            wx = fr[:, :, 1, :, :]
            m3 = maskj[:].rearrange("p (b k i) -> p b k i", b=NBLK, k=K2, i=R)

            a0 = work.tile([128, K2 * HI], F32)
            a0v = a0[:].rearrange("p (b k i) -> p b k i", b=NBLK, k=K2, i=R)
            nc.scalar.activation(a0v, wx, COPY, bias=1.0, scale=-1.0)
            nc.vector.tensor_tensor(a0v, a0v, m3, ALU.mult)
            a1 = work.tile([128, K2 * HI], F32)
            a1v = a1[:].rearrange("p (b k i) -> p b k i", b=NBLK, k=K2, i=R)
            nc.vector.tensor_tensor(a1v, wx, m3, ALU.mult)
            omy = work.tile([128, K2 * HI], F32)
            omyv = omy[:].rearrange("p (b k i) -> p b k i", b=NBLK, k=K2, i=R)
            nc.scalar.activation(omyv, wy, COPY, bias=1.0, scale=-1.0)

            # w4d: each mult writes both dups (dst pairs contiguous,
            # srcs 0-stride); all operands as [p, b(4), ki(144), two(2)]
            KI = K2 * R

            def dup_flat(t):   # contiguous [128, 576] tile
                ap = t[:]
                return bass.AP(ap.tensor, ap.offset,
                               [ap.ap[0], [KI, NBLK], [1, KI], [0, 2]])

            # wy view (b,k,i) strides (288,16,1) within frac's (b,two,k,i)
            wy_dup = bass.AP(wy.tensor, wy.offset,
                             [wy.ap[0], [2 * KI, NBLK], [1, KI], [0, 2]])

            for xc, yc, asrc in ((0, 0, a0), (0, 1, a0),
                                 (1, 0, a1), (1, 1, a1)):
                dst = w4v[:, :, :, :, xc, yc, :]
                dst = bass.AP(dst.tensor, dst.offset,
                              [dst.ap[0], [8 * KI, NBLK], [8, KI], [1, 2]])
                ysrc = dup_flat(omy) if yc == 0 else wy_dup
                nc.vector.tensor_tensor(dst, dup_flat(asrc), ysrc, ALU.mult)

        # ---- Phase 2: gather / combine / transpose / conv ----------------
        gpool = ctx.enter_context(tc.tile_pool(name="g", bufs=4))
        p4pool = ctx.enter_context(tc.tile_pool(name="p4", bufs=2))
        s2pool = ctx.enter_context(tc.tile_pool(name="s2", bufs=2))
        spool = ctx.enter_context(tc.tile_pool(name="s", bufs=2))
        stpool = ctx.enter_context(tc.tile_pool(name="st", bufs=2))
        obpool = ctx.enter_context(tc.tile_pool(name="ob", bufs=2))
        tpps = ctx.enter_context(tc.tile_pool(name="tp", bufs=2, space="PSUM"))
        outps = ctx.enter_context(tc.tile_pool(name="ops", bufs=1, space="PSUM"))

        idxs4 = idxs[:].rearrange("p (b k i jw) -> p b k i jw",
                                  b=NBLK, k=K2, i=R, jw=8)

        for b in range(NBLK):
            # out_ps columns: (parity, h, j) — even rows 0:1024, odd 1024:2048
            out_ps = outps.tile([64, R * W], F32)
            for k in range(K2):
                g = gpool.tile([128, R * 4 * C], F16)
                gv = g[:].rearrange("p (s e) -> p s e", s=R, e=4 * C)
                RSUB = 8          # rows per dma_gather call (1024 descs)
                for sub in range(R // RSUB):
                    nidx = RSUB * 128
                    nc.gpsimd.dma_gather(
                        gv[:, sub * RSUB:(sub + 1) * RSUB, :],
                        src_ap,
                        idxs4[:, b, k, sub * RSUB:(sub + 1) * RSUB, :],
                        nidx,
                        nidx,
                        elem_size=4 * C,
                        elem_step=2 * C,
                        queue_num=(b * K2 * 2 + k * 2 + sub) % 4,
                    )
                if b == 0 and k == 0:
                    phase1b()
                # weighted 4-corner combine, both operands inner step 1:
                # walk (i*xc*yc, c_hi, c_pair); weight strides (2, 0, 1)
                p4 = p4pool.tile([128, R * 4 * C], F16)
                wsl = w4v[:, b, k]
                w_b = bass.AP(
                    wsl.tensor, wsl.offset,
                    [wsl.ap[0], [2, R * 4], [0, C // 2], [1, 2]],
                )
                nc.vector.tensor_tensor(
                    p4[:].rearrange("p (icr chi c2) -> p icr chi c2",
                                    icr=R * 4, chi=C // 2, c2=2),
                    g[:].rearrange("p (icr chi c2) -> p icr chi c2",
                                   icr=R * 4, chi=C // 2, c2=2),
                    w_b, ALU.mult)
                if k == 0 and b + 1 < NBLK:
                    phase1a(b + 1)
                # sum x-corners (stride 2C), then y-corners (stride C)
                s2 = s2pool.tile([128, R * 2 * C], F16)
                p4v = p4[:].rearrange("p (i xc cc) -> p i xc cc",
                                      i=R, xc=2, cc=2 * C)
                nc.vector.tensor_tensor(
                    s2[:].rearrange("p (i cc) -> p i cc", i=R, cc=2 * C),
                    p4v[:, :, 0, :], p4v[:, :, 1, :], ALU.add)
                s = spool.tile([128, R * C], F16)
                s2v = s2[:].rearrange("p (i yc c) -> p i yc c", i=R, yc=2, c=C)
                sv = s[:].rearrange("p (i c) -> p i c", i=R, c=C)
                nc.vector.tensor_tensor(
                    sv, s2v[:, :, 0, :], s2v[:, :, 1, :], ALU.add)
                # transpose row-pairs: [128 j, (2i,64c)=128] -> [(2i,c), 128 j]
                st = stpool.tile([128, (R // 2) * 128], F16)
                tp = tpps.tile([128, (R // 2) * 128], F16)
                for h in range(R // 2):
                    nc.tensor.transpose(
                        tp[:, h * 128:(h + 1) * 128],
                        s[:, h * 128:(h + 1) * 128], ident[:])
                nc.scalar.copy(st[:], tp[:])
                # conv-accumulate; st[0:64]=even rows, st[64:128]=odd rows
                for par in range(2):
                    for c2 in range(2):
                        nc.tensor.matmul(
                            out_ps[:, par * 1024 + c2 * 512:
                                   par * 1024 + (c2 + 1) * 512],
                            wk2[64 * par:64 * par + 64, k * 64:(k + 1) * 64],
                            st[64 * par:64 * par + 64, c2 * 512:(c2 + 1) * 512],
                            start=(k == 0), stop=(k == K2 - 1))
            # unshuffle (parity, h, j) -> (i, j) during PSUM drain
            ob = obpool.tile([64, R * W], F32)
            obv = ob[:].rearrange("p (h par j) -> p h par j",
                                  h=R // 2, par=2, j=W)
            opv = out_ps[:].rearrange("p (par h j) -> p par h j",
                                      par=2, h=R // 2, j=W)
            nc.scalar.copy(obv[:, :, 0, :], opv[:, 0, :, :])
            nc.scalar.copy(obv[:, :, 1, :], opv[:, 1, :, :])
            nc.sync.dma_start(out_d[:, b * R * W:(b + 1) * R * W], ob[:])

    if not nc.is_finalized():
        nc.finalize()
    return nc


def _prep_core(x, offset, mask, wk2, core):
    n, half = core // 2, core % 2
    i0 = half * HI
    # clamp on host (device floor has no clamp); keeps gather slots in-pad
    offset = np.clip(offset, -CLAMP, CLAMP)

    # row-pair interleaved fp16 padded image P[y, x, yc, c]
    xp = np.zeros((PH, PW, C), np.float16)
    xp[PAD:PAD + H, PAD:PAD + W, :] = x[n].transpose(1, 2, 0)
    P = np.zeros((PH, PW, 2, C), np.float16)
    P[:, :, 0, :] = xp
    P[:-1, :, 1, :] = xp[1:]

    # j-major, block-major: offj (b, two, k, i16); maskj (b, k, i16)
    offj = offset[n, :, i0:i0 + HI, :].transpose(2, 0, 1)   # [j, 2K2, i]
    offj = offj.reshape(128, K2, 2, NBLK, R)                # ch = (k, two)
    offj = np.ascontiguousarray(
        offj.transpose(0, 3, 2, 1, 4)).reshape(128, 2 * K2 * HI)
    maskj = mask[n, :, i0:i0 + HI, :].transpose(2, 0, 1)    # [j, k, i]
    maskj = maskj.reshape(128, K2, NBLK, R)
    maskj = np.ascontiguousarray(
        maskj.transpose(0, 2, 1, 3)).reshape(128, K2 * HI)

    # wrapped layouts: partition p holds column j = 16*jw + (p%16)
    u = np.arange(128) % 16                       # [128]
    k = np.arange(K2)
    ki, kj = k // 3, k % 3
    i = np.arange(R)
    jw = np.arange(8)
    # offw[p, (b, two, k, i16, jw)]
    off5 = offset[n].reshape(K2, 2, H, W)         # [k, dy/dx, y, x]
    cols = (16 * jw[None, :] + u[:, None])        # [128, 8]
    offw = off5[:, :, i0:i0 + HI, :][:, :, :, cols]   # [k,2,i64,128,8]
    offw = offw.reshape(K2, 2, NBLK, R, 128, 8)
    offw = np.ascontiguousarray(
        offw.transpose(4, 2, 1, 0, 3, 5)).reshape(128, -1)

    # idxb[p, (b, k, i16, jw)] = slot index of the (y0, x0) corner
    b4 = np.arange(NBLK)
    base = ((i0 + b4[:, None, None, None] * R + i[None, None, :, None]
             + ki[None, :, None, None] - 1 + PAD) * PW
            + jw[None, None, None, :] * 16
            + kj[None, :, None, None] - 1 + PAD)          # [b, k, i, jw]
    idxb = base[None] + u[:, None, None, None, None]      # [128, b, k, i, jw]
    lo = idxb.min() - CLAMP * PW - CLAMP
    hi = idxb.max() + CLAMP * PW + CLAMP
    assert lo >= 0 and hi < NSLOT - 1, (lo, hi)
    idxb = idxb.reshape(128, -1).astype(np.float32)

    return {
        "xp": P.reshape(-1),
        "offj": offj,
        "maskj": maskj,
        "offw": np.ascontiguousarray(offw, np.float32),
        "idxb": idxb,
        "wk2": wk2,
        "ident": np.eye(128, dtype=np.float16),
    }


def _run(x, offset, mask, weight, trace=False, trace_kwargs=None):
    x = np.asarray(x, np.float32)
    offset = np.asarray(offset, np.float32)
    mask = np.asarray(mask, np.float32)
    weight = np.asarray(weight, np.float32)
    # wk2[p, k*64+o] = W[o, p%64, k], replicated on both partition halves
    wkco = weight.reshape(C, C, K2)               # [o, c, k]
    wk2 = np.ascontiguousarray(
        wkco.transpose(1, 2, 0)).reshape(C, K2 * C)   # [c, (k, o)]
    wk2 = np.concatenate([wk2, wk2], 0).astype(np.float16)

    if "nc" not in _CACHED:
        _CACHED["nc"] = build_nc()
    nc = _CACHED["nc"]
    in_maps = [_prep_core(x, offset, mask, wk2, core) for core in range(8)]
    if trace:
        res = run_bass_kernel_spmd(nc, in_maps, list(range(8)), trace=True,
                                   **(trace_kwargs or {}))
    else:
        res = run_bass_kernel_spmd(nc, in_maps, list(range(8)))
    out = np.empty((N, C, H, W), np.float32)
    for core in range(8):
        n, half = core // 2, core % 2
        out[n, :, half * HI:(half + 1) * HI, :] = (
            res.results[core]["out"].reshape(C, HI, W))
    return out, res


def kernel_traced(x, offset, mask, weight, trace=True, trace_kwargs=None):
    return _run(x, offset, mask, weight, trace=trace,
                trace_kwargs=trace_kwargs)


def kernel(x, offset, mask, weight):
    return _run(x, offset, mask, weight)[0]
